# revision 16
# baseline (speedup 1.0000x reference)
"""Background-embedding transformer layer (sparse attention) — Bass/Trainium2.

Self-contained: takes FULL unsharded inputs, returns FULL output.
Shapes (hardcoded per problem spec):
  bg [8, 4, 512], feature_values [131072, 512],
  feature_batch_idx/feature_level_idx [131072] int32, max_len = 16384.

Sharding: data-parallel over batch — core b owns batch b (feature rows
[b*16384, (b+1)*16384) since feature_batch_idx = arange // 16384).

Math restructuring (all equivalent to the reference):
  - RoPE on q and the level-dependent RoPE on k fold into 4 pre-rotated
    blocked query matrices A_L = blocked(R((b//2 - L) * freqs) @ q), so
    scores_L = fv @ (Wk @ A_L) contract raw fv directly (k is never
    materialized).  G_all = Wk @ [A_0|A_1|A_2|A_3] is [512, 128].
  - Level selection is an additive -1e9 mask, folded into the score matmul
    as 4 extra contraction rows (rank-1 expansion: mask = sum_L ind_L ⊗ m_L).
  - Softmax skips max-subtraction: scores ~ N(0,1) after the 1/8 scale, so
    exp is safe in fp32.
  - attn @ V becomes (attn^T @ fv) @ Wv: contract over keys first, so the
    only transpose needed in the hot loop is fv^T (done on TensorE).
"""

import numpy as np
import ml_dtypes

import concourse.bass as bass
import concourse.mybir as mybir
import concourse.tile as tile
from concourse import bass_utils
from concourse.masks import make_identity

# no S3 in the sandbox; keep NTFF artifacts local
bass_utils.upload_artifacts = lambda tmpdir: tmpdir

BSZ, NLVL, E, H, FFN = 8, 4, 512, 8, 2048
D = E // H
NNZ = 131072
NKEY = NNZ // BSZ          # 16384 keys per batch/core
NQ = NLVL                  # 4 queries per batch
CHUNK = 512                # keys per main-loop chunk
NCHUNK = NKEY // CHUNK     # 32
NEG = -1.0e9
NT4 = CHUNK // 128         # 4 key-subtiles per chunk

F32 = mybir.dt.float32
BF16 = mybir.dt.bfloat16
AF = mybir.ActivationFunctionType
bf16_np = ml_dtypes.bfloat16


def _build_bass():
    nc = bass.Bass()

    # ---- per-core inputs ----
    fv = nc.dram_tensor("fv", [NKEY, E], F32, kind="ExternalInput")
    bg = nc.dram_tensor("bg", [NQ, E], F32, kind="ExternalInput")
    maskT = nc.dram_tensor("maskT", [NLVL, NKEY], BF16, kind="ExternalInput")
    cs = nc.dram_tensor("cs", [NQ, 2 * NLVL * E], F32, kind="ExternalInput")
    # ---- shared inputs ----
    ind = nc.dram_tensor("ind", [NLVL, 128], BF16, kind="ExternalInput")
    sel = nc.dram_tensor("sel", [128, 32], BF16, kind="ExternalInput")
    wq = nc.dram_tensor("wq", [E, E], BF16, kind="ExternalInput")
    wkT = nc.dram_tensor("wkT", [E, E], BF16, kind="ExternalInput")
    wv = nc.dram_tensor("wv", [E, E], BF16, kind="ExternalInput")
    wo = nc.dram_tensor("wo", [E, E], BF16, kind="ExternalInput")
    w1 = nc.dram_tensor("w1", [E, FFN], BF16, kind="ExternalInput")
    w2 = nc.dram_tensor("w2", [FFN, E], BF16, kind="ExternalInput")
    b1t = nc.dram_tensor("b1t", [128, FFN // 128], F32, kind="ExternalInput")
    b2r = nc.dram_tensor("b2r", [NQ, E], F32, kind="ExternalInput")
    g1r = nc.dram_tensor("g1r", [NQ, E], F32, kind="ExternalInput")
    be1r = nc.dram_tensor("be1r", [NQ, E], F32, kind="ExternalInput")
    g2r = nc.dram_tensor("g2r", [NQ, E], F32, kind="ExternalInput")
    be2r = nc.dram_tensor("be2r", [NQ, E], F32, kind="ExternalInput")

    out = nc.dram_tensor("o", [NQ, E], F32, kind="ExternalOutput")

    ET = E // 128   # 4 e-tiles
    FT = FFN // 128  # 16 f-tiles

    def layernorm(nc, pool, x_sb, g_sb, b_sb, eps_ap):
        """x_sb [4, E] f32 -> fresh [4, E] f32 tile, ln with gamma/beta."""
        stats = pool.tile([NQ, nc.vector.BN_STATS_DIM], F32, tag="ln_stats")
        mv = pool.tile([NQ, nc.vector.BN_AGGR_DIM], F32, tag="ln_mv")
        nc.vector.bn_stats(out=stats[:], in_=x_sb[:])
        nc.vector.bn_aggr(out=mv[:], in_=stats[:])
        rstd = pool.tile([NQ, 1], F32, tag="ln_rstd")
        nc.scalar.activation(out=rstd[:], in_=mv[:, 1:2], func=AF.Sqrt,
                             bias=eps_ap)
        nc.vector.reciprocal(out=rstd[:], in_=rstd[:])
        xn = pool.tile([NQ, E], F32, tag="ln_out")
        nc.vector.tensor_scalar(
            out=xn[:], in0=x_sb[:], scalar1=mv[:, 0:1], scalar2=rstd[:],
            op0=mybir.AluOpType.subtract, op1=mybir.AluOpType.mult,
        )
        nc.vector.tensor_mul(out=xn[:], in0=xn[:], in1=g_sb[:])
        nc.vector.tensor_add(out=xn[:], in0=xn[:], in1=b_sb[:])
        return xn

    with tile.TileContext(nc) as tc, \
         tc.tile_pool(name="const", bufs=1) as cpool:
        # persistent constants / weights (bf16 via SWDGE cast-DMA)
        id_bf = cpool.tile([128, 128], BF16)
        make_identity(nc, id_bf[:])
        ones_sb = cpool.tile([128, 1], BF16)
        nc.vector.memset(ones_sb[:], 1.0)

        ind_sb = cpool.tile([NLVL, 128], BF16)
        nc.sync.dma_start(ind_sb[:], ind[:])
        sel_sb = cpool.tile([128, 32], BF16)
        nc.sync.dma_start(sel_sb[:], sel[:])
        maskT_sb = cpool.tile([NLVL, NKEY], BF16)
        nc.sync.dma_start(maskT_sb[:], maskT[:])

        def load_w_bf16(dram, rows, cols):
            t = cpool.tile([128, rows // 128, cols], BF16)
            nc.sync.dma_start(
                t[:], dram[:, :].rearrange("(t p) n -> p t n", p=128)
            )
            return t

        wq_sb = load_w_bf16(wq, E, E)
        wkT_sb = load_w_bf16(wkT, E, E)
        wv_sb = load_w_bf16(wv, E, E)
        wo_sb = load_w_bf16(wo, E, E)
        w1_sb = load_w_bf16(w1, E, FFN)
        w2_sb = load_w_bf16(w2, FFN, E)
        b1t_sb = cpool.tile([128, FT], F32)
        nc.sync.dma_start(b1t_sb[:], b1t[:])

        smalls = {}
        for nm, dr in [("bg", bg), ("b2r", b2r), ("g1r", g1r), ("be1r", be1r),
                       ("g2r", g2r), ("be2r", be2r)]:
            t = cpool.tile([NQ, E], F32, tag=f"sm_{nm}")
            nc.sync.dma_start(t[:], dr[:])
            smalls[nm] = t
        cs_sb = cpool.tile([NQ, 2 * NLVL, E], F32)
        nc.sync.dma_start(cs_sb[:], cs[:, :].rearrange("p (t e) -> p t e", e=E))
        eps_sb = cpool.tile([NQ, 1], F32)
        nc.vector.memset(eps_sb[:], 1e-5)

        # ---------------- prologue: q-side -> G_all ----------------
        A_sb = cpool.tile([128, ET, 128], BF16)
        nc.vector.memset(A_sb[:], 0.0)
        G_sb = cpool.tile([128, ET, 128], BF16)

        # loop SBUF pools opened first: their addresses sit below the prologue
        # pools on the stack allocator, so fv DMAs have no WAR on prologue
        loop_sbuf = [tc.tile_pool(name="fvp", bufs=4),
                     tc.tile_pool(name="fvTp", bufs=3),
                     tc.tile_pool(name="exp", bufs=4)]
        fvp, fvTp, expp = [p.__enter__() for p in loop_sbuf]

        with tc.tile_pool(name="pre", bufs=2) as pre, \
             tc.tile_pool(name="preps", bufs=2, space="PSUM") as preps:
            xln = layernorm(nc, pre, smalls["bg"], smalls["g1r"], smalls["be1r"], eps_sb[:])
            x_bf = pre.tile([NQ, E], BF16)
            nc.vector.tensor_copy(x_bf[:], xln[:])
            xT_sb = pre.tile([128, ET, NQ], BF16)
            for e in range(ET):
                tp = preps.tile([128, NQ], BF16, tag="tr4")
                nc.tensor.transpose(tp[:], x_bf[:, e * 128:(e + 1) * 128],
                                    id_bf[:NQ, :NQ])
                nc.scalar.copy(xT_sb[:, e, :], tp[:])
            q_ps = preps.tile([NQ, E], F32, tag="qps")
            for e in range(ET):
                nc.tensor.matmul(q_ps[:], xT_sb[:, e, :], wq_sb[:, e, :],
                                 start=(e == 0), stop=(e == ET - 1))
            q_sb = pre.tile([NQ, E], F32)
            nc.vector.tensor_copy(q_sb[:], q_ps[:])
            qsw = pre.tile([NQ, E], F32)
            q3 = q_sb.rearrange("p (n two) -> p n two", two=2)
            w3 = qsw.rearrange("p (n two) -> p n two", two=2)
            nc.vector.tensor_copy(w3[:, :, 0:1], q3[:, :, 1:2])
            nc.vector.tensor_copy(w3[:, :, 1:2], q3[:, :, 0:1])
            for L in range(NLVL):
                qr = pre.tile([NQ, E], F32, tag="qr")
                tmp = pre.tile([NQ, E], F32, tag="qrtmp")
                nc.vector.tensor_mul(qr[:], q_sb[:], cs_sb[:, L, :])
                nc.vector.tensor_mul(tmp[:], qsw[:], cs_sb[:, NLVL + L, :])
                nc.vector.tensor_add(qr[:], qr[:], tmp[:])
                qr_bf = pre.tile([NQ, E], BF16, tag="qrbf")
                nc.vector.tensor_copy(qr_bf[:], qr[:])
                for e in range(ET):
                    tp = preps.tile([128, NQ], BF16, tag="tr4")
                    nc.tensor.transpose(tp[:], qr_bf[:, e * 128:(e + 1) * 128],
                                        id_bf[:NQ, :NQ])
                    c0 = L * 32 + 8 * e
                    nc.scalar.copy(A_sb[0:64, e, c0:c0 + 4], tp[0:64, :])
                    nc.scalar.copy(A_sb[64:128, e, c0 + 4:c0 + 8], tp[64:128, :])
            for m in range(ET):
                gp = preps.tile([128, 128], F32, tag="gps")
                for k in range(ET):
                    nc.tensor.matmul(
                        gp[:], wkT_sb[:, k, m * 128:(m + 1) * 128],
                        A_sb[:, k, :], start=(k == 0), stop=(k == ET - 1))
                nc.scalar.copy(G_sb[:, m, :], gp[:])

        # ---------------- main loop over key chunks ----------------
        T_sb = cpool.tile([128, E], BF16)
        Dn_sb = cpool.tile([128, 1], BF16)

        with tc.tile_pool(name="accps", bufs=1, space="PSUM") as accps:
            T_ps = accps.tile([128, E], F32)
            Dn_ps = accps.tile([128, 1], F32)

            with tc.tile_pool(name="trps", bufs=2, space="PSUM") as trps, \
                 tc.tile_pool(name="sps", bufs=2, space="PSUM") as sps:
                for c in range(NCHUNK):
                    fvb = fvp.tile([128, NT4, E], BF16, tag="fvb")
                    nc.gpsimd.dma_start(
                        fvb[:],
                        fv[c * CHUNK:(c + 1) * CHUNK, :].rearrange(
                            "(p t) e -> p t e", p=128))
                    fvT = fvTp.tile([128, NT4 * ET, 128], BF16, tag="fvT")
                    for t4 in range(NT4):
                        trp = trps.tile([128, ET, 128], BF16, tag="trp")
                        for e in range(ET):
                            nc.tensor.transpose(
                                trp[:, e, :], fvb[:, t4, e * 128:(e + 1) * 128],
                                id_bf[:])
                        if t4 < 3:
                            nc.vector.tensor_copy(
                                fvT[:, t4 * ET:(t4 + 1) * ET, :], trp[:])
                        else:
                            nc.scalar.copy(
                                fvT[:, t4 * ET:(t4 + 1) * ET, :], trp[:])
                    for t4 in range(NT4):
                        sp = sps.tile([128, 128], F32, tag="sp")
                        for e in range(ET):
                            nc.tensor.matmul(
                                sp[:], fvT[:, t4 * ET + e, :], G_sb[:, e, :],
                                start=(e == 0), stop=False)
                        k0 = c * CHUNK
                        nc.tensor.matmul(
                            sp[:], maskT_sb[:, k0 + t4:k0 + CHUNK:NT4],
                            ind_sb[:], start=False, stop=True)
                        ex = expp.tile([128, 128], BF16, tag="ex")
                        nc.scalar.activation(out=ex[:], in_=sp[:], func=AF.Exp,
                                             scale=0.125)
                        first = (c == 0 and t4 == 0)
                        last = (c == NCHUNK - 1 and t4 == NT4 - 1)
                        nc.tensor.matmul(T_ps[:], ex[:], fvb[:, t4, :],
                                         start=first, stop=last)
                        nc.tensor.matmul(Dn_ps[:], ex[:], ones_sb[:],
                                         start=first, stop=last)

            nc.vector.tensor_copy(T_sb[:], T_ps[:])
            nc.vector.tensor_copy(Dn_sb[:], Dn_ps[:])
        for p in reversed(loop_sbuf):
            p.__exit__(None, None, None)

        # ---------------- tail ----------------
        with tc.tile_pool(name="tail", bufs=2) as tl, \
             tc.tile_pool(name="tailps", bufs=3, space="PSUM") as tlps:
            tsel = tlps.tile([32, E], F32, tag="ps")
            nc.tensor.matmul(tsel[:], sel_sb[:], T_sb[:], start=True, stop=True)
            dsel = tlps.tile([32, 1], F32, tag="ps")
            nc.tensor.matmul(dsel[:], sel_sb[:], Dn_sb[:], start=True, stop=True)
            rinv = tl.tile([32, 1], F32)
            nc.vector.tensor_copy(rinv[:], dsel[:])
            nc.vector.reciprocal(rinv[:], rinv[:])
            Tn = tl.tile([32, E], BF16)
            nc.vector.tensor_scalar(
                out=Tn[:], in0=tsel[:], scalar1=rinv[:], scalar2=None,
                op0=mybir.AluOpType.mult)

            TnT = tl.tile([128, ET, 32], BF16)
            for e in range(ET):
                tp = tlps.tile([128, 32], BF16, tag="ps")
                nc.tensor.transpose(tp[:], Tn[:, e * 128:(e + 1) * 128],
                                    id_bf[:32, :32])
                nc.scalar.copy(TnT[:, e, :], tp[:])
            ov_ps = tlps.tile([32, E], F32, tag="ps")
            for e in range(ET):
                nc.tensor.matmul(ov_ps[:], TnT[:, e, :], wv_sb[:, e, :],
                                 start=(e == 0), stop=(e == ET - 1))
            ov_bf = tl.tile([32, E], BF16)
            nc.vector.tensor_copy(ov_bf[:], ov_ps[:])
            ovT = tl.tile([128, ET, 32], BF16)
            for e in range(ET):
                tp = tlps.tile([128, 32], BF16, tag="ps")
                nc.tensor.transpose(tp[:], ov_bf[:, e * 128:(e + 1) * 128],
                                    id_bf[:32, :32])
                nc.scalar.copy(ovT[:, e, :], tp[:])
            oT = tl.tile([128, ET, NQ], BF16)
            for e in range(ET):
                nc.vector.tensor_copy(oT[0:64, e, :],
                                      ovT[0:64, e, 8 * e:8 * e + 4])
                nc.vector.tensor_copy(oT[64:128, e, :],
                                      ovT[64:128, e, 8 * e + 4:8 * e + 8])
            o2_ps = tlps.tile([NQ, E], F32, tag="ps")
            for e in range(ET):
                nc.tensor.matmul(o2_ps[:], oT[:, e, :], wo_sb[:, e, :],
                                 start=(e == 0), stop=(e == ET - 1))
            o2 = tl.tile([NQ, E], F32)
            nc.vector.tensor_add(o2[:], o2_ps[:], smalls["bg"][:])

            z = layernorm(nc, tl, o2, smalls["g2r"], smalls["be2r"], eps_sb[:])
            z_bf = tl.tile([NQ, E], BF16)
            nc.vector.tensor_copy(z_bf[:], z[:])
            zT = tl.tile([128, ET, NQ], BF16)
            for e in range(ET):
                tp = tlps.tile([128, NQ], BF16, tag="ps")
                nc.tensor.transpose(tp[:], z_bf[:, e * 128:(e + 1) * 128],
                                    id_bf[:NQ, :NQ])
                nc.scalar.copy(zT[:, e, :], tp[:])
            y1r = tl.tile([128, FT, NQ], BF16)
            for ft in range(FT):
                yp = tlps.tile([128, NQ], F32, tag="ps")
                for e in range(ET):
                    nc.tensor.matmul(
                        yp[:], w1_sb[:, e, ft * 128:(ft + 1) * 128],
                        zT[:, e, :], start=(e == 0), stop=(e == ET - 1))
                nc.scalar.activation(out=y1r[:, ft, :], in_=yp[:], func=AF.Relu,
                                     bias=b1t_sb[:, ft:ft + 1])
            y2_ps = tlps.tile([NQ, E], F32, tag="ps")
            for ft in range(FT):
                nc.tensor.matmul(y2_ps[:], y1r[:, ft, :], w2_sb[:, ft, :],
                                 start=(ft == 0), stop=(ft == FT - 1))
            res = tl.tile([NQ, E], F32)
            nc.vector.tensor_add(res[:], y2_ps[:], o2[:])
            nc.vector.tensor_add(res[:], res[:], smalls["b2r"][:])
            nc.sync.dma_start(out[:, :], res[:])

    _split_multi_waits(nc)
    return nc


def _host_prep(bg, feature_values, feature_batch_idx, feature_level_idx,
               Wq, Wkv, Wo, rope_freqs, ln_attn_g, ln_attn_b,
               ln_ffn_g, ln_ffn_b, W1, b1, W2, b2, max_len):
    """Index/layout-only host prep -> per-core in_maps."""
    bg = np.asarray(bg, np.float32)
    fv = np.ascontiguousarray(np.asarray(feature_values, np.float32))
    lvl = np.asarray(feature_level_idx).astype(np.int64)
    Wq = np.asarray(Wq, np.float32)
    Wkv = np.asarray(Wkv, np.float32)
    Wo = np.asarray(Wo, np.float32)
    freqs = np.asarray(rope_freqs, np.float32)      # [H, D/2]
    W1 = np.asarray(W1, np.float32)
    W2 = np.asarray(W2, np.float32)
    b1 = np.asarray(b1, np.float32)
    b2 = np.asarray(b2, np.float32)

    wk = np.ascontiguousarray(Wkv[:, :E])
    wv = np.ascontiguousarray(Wkv[:, E:])
    wkT = np.ascontiguousarray(wk.T)

    # per-e rope table entries
    e_idx = np.arange(E)
    h_of_e = e_idx // D
    p_of_e = (e_idx % D) // 2
    f_of_e = freqs[h_of_e, p_of_e]                  # [E]
    sign = np.where(e_idx % 2 == 0, -1.0, 1.0).astype(np.float32)

    ind = np.zeros((NLVL, 128), bf16_np)
    for L in range(NLVL):
        ind[L, L * 32:(L + 1) * 32] = 1.0
    selm = np.zeros((128, 32), bf16_np)
    for L in range(NLVL):
        selm[L * 32:(L + 1) * 32, :] = np.eye(32, dtype=bf16_np)

    b1t = np.ascontiguousarray(b1.reshape(FFN // 128, 128).T).astype(np.float32)

    def rep4(v):
        return np.ascontiguousarray(
            np.broadcast_to(np.asarray(v, np.float32)[None, :], (NQ, E)))

    shared = {
        "ind": ind, "sel": selm,
        "wq": Wq.astype(bf16_np), "wkT": wkT.astype(bf16_np),
        "wv": wv.astype(bf16_np), "wo": Wo.astype(bf16_np),
        "w1": W1.astype(bf16_np), "w2": W2.astype(bf16_np), "b1t": b1t,
        "b2r": rep4(b2), "g1r": rep4(ln_attn_g), "be1r": rep4(ln_attn_b),
        "g2r": rep4(ln_ffn_g), "be2r": rep4(ln_ffn_b),
    }

    in_maps = []
    for b in range(BSZ):
        lv = lvl[b * NKEY:(b + 1) * NKEY]
        maskT = np.where(lv[None, :] == np.arange(NLVL)[:, None],
                         np.float32(0.0), np.float32(NEG)).astype(bf16_np)
        qpos = float(b // 2)
        cs = np.empty((NQ, 2 * NLVL, E), np.float32)
        for L in range(NLVL):
            ang = (qpos - L) * f_of_e
            cs[:, L, :] = np.cos(ang)[None, :]
            cs[:, NLVL + L, :] = (sign * np.sin(ang))[None, :]
        cs = cs.reshape(NQ, 2 * NLVL * E)
        m = {
            "fv": fv[b * NKEY:(b + 1) * NKEY],
            "bg": np.ascontiguousarray(bg[b]),
            "maskT": maskT,
            "cs": cs,
        }
        m.update(shared)
        in_maps.append(m)
    return in_maps


def _split_multi_waits(nc):
    """This walrus build caps sync-wait commands at 1 per instruction; Tile
    emits more.  Hoist extra waits onto injected same-engine NOPs placed
    immediately before the waiting instruction (same per-engine order, so
    semantics are identical)."""
    k = 0
    for f in nc.m.functions:
        for bb in f.blocks:
            new_list = []
            for ins in bb.instructions:
                si = ins.sync_info
                if si is not None and si.on_wait and len(si.on_wait) > 1:
                    waits = list(si.on_wait)
                    for w in waits[:-1]:
                        nop = mybir.InstNoOp(
                            name=f"I-sw{k}", ins=[], outs=[], nofuse=True)
                        k += 1
                        nop.engine = ins.engine
                        nop.sync_info = mybir.SyncInfo(
                            on_wait=[w], on_update=[])
                        new_list.append(nop)
                    si.on_wait.clear()
                    si.on_wait.append(waits[-1])
                new_list.append(ins)
            bb.instructions[:] = new_list

_NC_CACHE = None


def _get_nc():
    global _NC_CACHE
    if _NC_CACHE is None:
        _NC_CACHE = _build_bass()
    return _NC_CACHE


def _run(inputs, trace=False):
    nc = _get_nc()
    in_maps = _host_prep(**inputs)
    res = bass_utils.run_bass_kernel_spmd(
        nc, in_maps, core_ids=list(range(BSZ)), trace=trace)
    out = np.stack([res.results[b]["o"] for b in range(BSZ)], axis=0)
    return out.astype(np.float32), res


def kernel(**inputs) -> np.ndarray:
    out, _ = _run(inputs, trace=False)
    return out


# revision 17
# speedup vs baseline: 1.0665x; 1.0665x over previous
"""Background-embedding transformer layer (sparse attention) — Bass/Trainium2.

Self-contained: takes FULL unsharded inputs, returns FULL output.
Shapes (hardcoded per problem spec):
  bg [8, 4, 512], feature_values [131072, 512],
  feature_batch_idx/feature_level_idx [131072] int32, max_len = 16384.

Sharding: data-parallel over batch — core b owns batch b (feature rows
[b*16384, (b+1)*16384) since feature_batch_idx = arange // 16384).

Math restructuring (all equivalent to the reference):
  - RoPE on q and the level-dependent RoPE on k fold into 4 pre-rotated
    blocked query matrices A_L = blocked(R((b//2 - L) * freqs) @ q), so
    scores_L = fv @ (Wk @ A_L) contract raw fv directly (k is never
    materialized).  G_all = Wk @ [A_0|A_1|A_2|A_3] is [512, 128].
  - Level selection is an additive -1e9 mask, folded into the score matmul
    as 4 extra contraction rows (rank-1 expansion: mask = sum_L ind_L ⊗ m_L).
  - Softmax skips max-subtraction: scores ~ N(0,1) after the 1/8 scale, so
    exp is safe in fp32.
  - attn @ V becomes (attn^T @ fv) @ Wv: contract over keys first, so the
    only transpose needed in the hot loop is fv^T (done on TensorE).
"""

import numpy as np
import ml_dtypes

import concourse.bass as bass
import concourse.mybir as mybir
import concourse.tile as tile
from concourse import bass_utils
from concourse.masks import make_identity

# no S3 in the sandbox; keep NTFF artifacts local
bass_utils.upload_artifacts = lambda tmpdir: tmpdir

BSZ, NLVL, E, H, FFN = 8, 4, 512, 8, 2048
D = E // H
NNZ = 131072
NKEY = NNZ // BSZ          # 16384 keys per batch/core
NQ = NLVL                  # 4 queries per batch
CHUNK = 512                # keys per main-loop chunk
NCHUNK = NKEY // CHUNK     # 32
NEG = -1.0e9
NT4 = CHUNK // 128         # 4 key-subtiles per chunk

F32 = mybir.dt.float32
BF16 = mybir.dt.bfloat16
AF = mybir.ActivationFunctionType
bf16_np = ml_dtypes.bfloat16


def _build_bass():
    nc = bass.Bass()

    # ---- per-core inputs ----
    fv = nc.dram_tensor("fv", [NKEY, E], F32, kind="ExternalInput")
    bg = nc.dram_tensor("bg", [NQ, E], F32, kind="ExternalInput")
    maskT = nc.dram_tensor("maskT", [NLVL, NKEY], BF16, kind="ExternalInput")
    cs = nc.dram_tensor("cs", [NQ, 2 * NLVL * E], F32, kind="ExternalInput")
    # ---- shared inputs ----
    ind = nc.dram_tensor("ind", [NLVL, 128], BF16, kind="ExternalInput")
    sel = nc.dram_tensor("sel", [128, 32], BF16, kind="ExternalInput")
    wq = nc.dram_tensor("wq", [E, E], BF16, kind="ExternalInput")
    wkT = nc.dram_tensor("wkT", [E, E], BF16, kind="ExternalInput")
    wv = nc.dram_tensor("wv", [E, E], BF16, kind="ExternalInput")
    wo = nc.dram_tensor("wo", [E, E], BF16, kind="ExternalInput")
    w1 = nc.dram_tensor("w1", [E, FFN], BF16, kind="ExternalInput")
    w2 = nc.dram_tensor("w2", [FFN, E], BF16, kind="ExternalInput")
    b1t = nc.dram_tensor("b1t", [128, FFN // 128], F32, kind="ExternalInput")
    b2r = nc.dram_tensor("b2r", [NQ, E], F32, kind="ExternalInput")
    g1r = nc.dram_tensor("g1r", [NQ, E], F32, kind="ExternalInput")
    be1r = nc.dram_tensor("be1r", [NQ, E], F32, kind="ExternalInput")
    g2r = nc.dram_tensor("g2r", [NQ, E], F32, kind="ExternalInput")
    be2r = nc.dram_tensor("be2r", [NQ, E], F32, kind="ExternalInput")

    out = nc.dram_tensor("o", [NQ, E], F32, kind="ExternalOutput")

    ET = E // 128   # 4 e-tiles
    FT = FFN // 128  # 16 f-tiles

    def layernorm(nc, pool, x_sb, g_sb, b_sb, eps_ap):
        """x_sb [4, E] f32 -> fresh [4, E] f32 tile, ln with gamma/beta."""
        stats = pool.tile([NQ, nc.vector.BN_STATS_DIM], F32, tag="ln_stats")
        mv = pool.tile([NQ, nc.vector.BN_AGGR_DIM], F32, tag="ln_mv")
        nc.vector.bn_stats(out=stats[:], in_=x_sb[:])
        nc.vector.bn_aggr(out=mv[:], in_=stats[:])
        rstd = pool.tile([NQ, 1], F32, tag="ln_rstd")
        nc.scalar.activation(out=rstd[:], in_=mv[:, 1:2], func=AF.Sqrt,
                             bias=eps_ap)
        nc.vector.reciprocal(out=rstd[:], in_=rstd[:])
        xn = pool.tile([NQ, E], F32, tag="ln_out")
        nc.vector.tensor_scalar(
            out=xn[:], in0=x_sb[:], scalar1=mv[:, 0:1], scalar2=rstd[:],
            op0=mybir.AluOpType.subtract, op1=mybir.AluOpType.mult,
        )
        nc.vector.tensor_mul(out=xn[:], in0=xn[:], in1=g_sb[:])
        nc.vector.tensor_add(out=xn[:], in0=xn[:], in1=b_sb[:])
        return xn

    with tile.TileContext(nc) as tc, \
         tc.tile_pool(name="const", bufs=1) as cpool:
        # persistent constants / weights (bf16 via SWDGE cast-DMA)
        id_bf = cpool.tile([128, 128], BF16)
        make_identity(nc, id_bf[:])
        ones_sb = cpool.tile([128, 1], BF16)
        nc.vector.memset(ones_sb[:], 1.0)

        ind_sb = cpool.tile([NLVL, 128], BF16)
        nc.sync.dma_start(ind_sb[:], ind[:])
        sel_sb = cpool.tile([128, 32], BF16)
        nc.sync.dma_start(sel_sb[:], sel[:])
        maskT_sb = cpool.tile([NLVL, NKEY], BF16)
        nc.sync.dma_start(maskT_sb[:], maskT[:])

        def load_w_bf16(dram, rows, cols):
            t = cpool.tile([128, rows // 128, cols], BF16)
            nc.sync.dma_start(
                t[:], dram[:, :].rearrange("(t p) n -> p t n", p=128)
            )
            return t

        wq_sb = load_w_bf16(wq, E, E)
        wkT_sb = load_w_bf16(wkT, E, E)
        wv_sb = cpool.tile([128, ET, E], BF16)
        wo_sb = cpool.tile([128, ET, E], BF16)
        w1_sb = cpool.tile([128, ET, FFN], BF16)
        w2_sb = cpool.tile([128, FT, E], BF16)
        b1t_sb = cpool.tile([128, FT], F32)
        deferred_loads = []
        for dst, dram, nt in [(wv_sb, wv, ET), (wo_sb, wo, ET),
                              (w1_sb, w1, ET), (w2_sb, w2, FT)]:
            src = dram[:, :].rearrange("(t p) n -> p t n", p=128)
            for t in range(nt):
                deferred_loads.append((dst[:, t, :], src[:, t, :]))
        deferred_loads.append((b1t_sb[:], b1t[:]))

        smalls = {}
        for nm, dr in [("bg", bg), ("b2r", b2r), ("g1r", g1r), ("be1r", be1r),
                       ("g2r", g2r), ("be2r", be2r)]:
            t = cpool.tile([NQ, E], F32, tag=f"sm_{nm}")
            nc.sync.dma_start(t[:], dr[:])
            smalls[nm] = t
        cs_sb = cpool.tile([NQ, 2 * NLVL, E], F32)
        nc.sync.dma_start(cs_sb[:], cs[:, :].rearrange("p (t e) -> p t e", e=E))
        eps_sb = cpool.tile([NQ, 1], F32)
        nc.vector.memset(eps_sb[:], 1e-5)

        # ---------------- prologue: q-side -> G_all ----------------
        A_sb = cpool.tile([128, ET, 128], BF16)
        nc.vector.memset(A_sb[:], 0.0)
        G_sb = cpool.tile([128, ET, 128], BF16)

        # loop SBUF pools opened first: their addresses sit below the prologue
        # pools on the stack allocator, so fv DMAs have no WAR on prologue
        loop_sbuf = [tc.tile_pool(name="fvp", bufs=4),
                     tc.tile_pool(name="fvTp", bufs=3),
                     tc.tile_pool(name="exp", bufs=4)]
        fvp, fvTp, expp = [p.__enter__() for p in loop_sbuf]

        with tc.tile_pool(name="pre", bufs=2) as pre, \
             tc.tile_pool(name="preps", bufs=2, space="PSUM") as preps:
            xln = layernorm(nc, pre, smalls["bg"], smalls["g1r"], smalls["be1r"], eps_sb[:])
            x_bf = pre.tile([NQ, E], BF16)
            nc.vector.tensor_copy(x_bf[:], xln[:])
            xT_sb = pre.tile([128, ET, NQ], BF16)
            for e in range(ET):
                tp = preps.tile([128, NQ], BF16, tag="tr4")
                nc.tensor.transpose(tp[:], x_bf[:, e * 128:(e + 1) * 128],
                                    id_bf[:NQ, :NQ])
                nc.scalar.copy(xT_sb[:, e, :], tp[:])
            q_ps = preps.tile([NQ, E], F32, tag="qps")
            for e in range(ET):
                nc.tensor.matmul(q_ps[:], xT_sb[:, e, :], wq_sb[:, e, :],
                                 start=(e == 0), stop=(e == ET - 1))
            q_sb = pre.tile([NQ, E], F32)
            nc.vector.tensor_copy(q_sb[:], q_ps[:])
            qsw = pre.tile([NQ, E], F32)
            q3 = q_sb.rearrange("p (n two) -> p n two", two=2)
            w3 = qsw.rearrange("p (n two) -> p n two", two=2)
            nc.vector.tensor_copy(w3[:, :, 0:1], q3[:, :, 1:2])
            nc.vector.tensor_copy(w3[:, :, 1:2], q3[:, :, 0:1])
            for L in range(NLVL):
                qr = pre.tile([NQ, E], F32, tag="qr")
                tmp = pre.tile([NQ, E], F32, tag="qrtmp")
                nc.vector.tensor_mul(qr[:], q_sb[:], cs_sb[:, L, :])
                nc.vector.tensor_mul(tmp[:], qsw[:], cs_sb[:, NLVL + L, :])
                nc.vector.tensor_add(qr[:], qr[:], tmp[:])
                qr_bf = pre.tile([NQ, E], BF16, tag="qrbf")
                nc.vector.tensor_copy(qr_bf[:], qr[:])
                for e in range(ET):
                    tp = preps.tile([128, NQ], BF16, tag="tr4")
                    nc.tensor.transpose(tp[:], qr_bf[:, e * 128:(e + 1) * 128],
                                        id_bf[:NQ, :NQ])
                    c0 = L * 32 + 8 * e
                    nc.scalar.copy(A_sb[0:64, e, c0:c0 + 4], tp[0:64, :])
                    nc.scalar.copy(A_sb[64:128, e, c0 + 4:c0 + 8], tp[64:128, :])
            for m in range(ET):
                gp = preps.tile([128, 128], F32, tag="gps")
                for k in range(ET):
                    nc.tensor.matmul(
                        gp[:], wkT_sb[:, k, m * 128:(m + 1) * 128],
                        A_sb[:, k, :], start=(k == 0), stop=(k == ET - 1))
                nc.scalar.copy(G_sb[:, m, :], gp[:])

        # ---------------- main loop over key chunks ----------------
        T_sb = cpool.tile([128, E], BF16)
        Dn_sb = cpool.tile([128, 1], BF16)

        with tc.tile_pool(name="accps", bufs=1, space="PSUM") as accps:
            T_ps = accps.tile([128, E], F32)
            Dn_ps = accps.tile([128, 1], F32)

            with tc.tile_pool(name="trps", bufs=2, space="PSUM") as trps, \
                 tc.tile_pool(name="sps", bufs=2, space="PSUM") as sps:
                for c in range(NCHUNK):
                    fvb = fvp.tile([128, NT4, E], BF16, tag="fvb")
                    nc.gpsimd.dma_start(
                        fvb[:],
                        fv[c * CHUNK:(c + 1) * CHUNK, :].rearrange(
                            "(p t) e -> p t e", p=128))
                    if c < len(deferred_loads):
                        dst, src = deferred_loads[c]
                        nc.sync.dma_start(dst, src)
                    fvT = fvTp.tile([128, NT4 * ET, 128], BF16, tag="fvT")
                    for t4 in range(NT4):
                        trp = trps.tile([128, ET, 128], BF16, tag="trp")
                        for e in range(ET):
                            nc.tensor.transpose(
                                trp[:, e, :], fvb[:, t4, e * 128:(e + 1) * 128],
                                id_bf[:])
                        if t4 < 3:
                            nc.vector.tensor_copy(
                                fvT[:, t4 * ET:(t4 + 1) * ET, :], trp[:])
                        else:
                            nc.scalar.copy(
                                fvT[:, t4 * ET:(t4 + 1) * ET, :], trp[:])
                    for t4 in range(NT4):
                        sp = sps.tile([128, 128], F32, tag="sp")
                        for e in range(ET):
                            nc.tensor.matmul(
                                sp[:], fvT[:, t4 * ET + e, :], G_sb[:, e, :],
                                start=(e == 0), stop=False)
                        k0 = c * CHUNK
                        nc.tensor.matmul(
                            sp[:], maskT_sb[:, k0 + t4:k0 + CHUNK:NT4],
                            ind_sb[:], start=False, stop=True)
                        ex = expp.tile([128, 128], BF16, tag="ex")
                        nc.scalar.activation(out=ex[:], in_=sp[:], func=AF.Exp,
                                             scale=0.125)
                        first = (c == 0 and t4 == 0)
                        last = (c == NCHUNK - 1 and t4 == NT4 - 1)
                        nc.tensor.matmul(T_ps[:], ex[:], fvb[:, t4, :],
                                         start=first, stop=last)
                        nc.tensor.matmul(Dn_ps[:], ex[:], ones_sb[:],
                                         start=first, stop=last)

            nc.vector.tensor_copy(T_sb[:], T_ps[:])
            nc.vector.tensor_copy(Dn_sb[:], Dn_ps[:])
        for p in reversed(loop_sbuf):
            p.__exit__(None, None, None)

        # ---------------- tail ----------------
        with tc.tile_pool(name="tail", bufs=2) as tl, \
             tc.tile_pool(name="tailps", bufs=3, space="PSUM") as tlps:
            tsel = tlps.tile([32, E], F32, tag="ps")
            nc.tensor.matmul(tsel[:], sel_sb[:], T_sb[:], start=True, stop=True)
            dsel = tlps.tile([32, 1], F32, tag="ps")
            nc.tensor.matmul(dsel[:], sel_sb[:], Dn_sb[:], start=True, stop=True)
            rinv = tl.tile([32, 1], F32)
            nc.vector.tensor_copy(rinv[:], dsel[:])
            nc.vector.reciprocal(rinv[:], rinv[:])
            Tn = tl.tile([32, E], BF16)
            nc.vector.tensor_scalar(
                out=Tn[:], in0=tsel[:], scalar1=rinv[:], scalar2=None,
                op0=mybir.AluOpType.mult)

            TnT = tl.tile([128, ET, 32], BF16)
            for e in range(ET):
                tp = tlps.tile([128, 32], BF16, tag="ps")
                nc.tensor.transpose(tp[:], Tn[:, e * 128:(e + 1) * 128],
                                    id_bf[:32, :32])
                nc.scalar.copy(TnT[:, e, :], tp[:])
            ov_ps = tlps.tile([32, E], F32, tag="ps")
            for e in range(ET):
                nc.tensor.matmul(ov_ps[:], TnT[:, e, :], wv_sb[:, e, :],
                                 start=(e == 0), stop=(e == ET - 1))
            ov_bf = tl.tile([32, E], BF16)
            nc.vector.tensor_copy(ov_bf[:], ov_ps[:])
            ovT = tl.tile([128, ET, 32], BF16)
            for e in range(ET):
                tp = tlps.tile([128, 32], BF16, tag="ps")
                nc.tensor.transpose(tp[:], ov_bf[:, e * 128:(e + 1) * 128],
                                    id_bf[:32, :32])
                nc.scalar.copy(ovT[:, e, :], tp[:])
            oT = tl.tile([128, ET, NQ], BF16)
            for e in range(ET):
                nc.vector.tensor_copy(oT[0:64, e, :],
                                      ovT[0:64, e, 8 * e:8 * e + 4])
                nc.vector.tensor_copy(oT[64:128, e, :],
                                      ovT[64:128, e, 8 * e + 4:8 * e + 8])
            o2_ps = tlps.tile([NQ, E], F32, tag="ps")
            for e in range(ET):
                nc.tensor.matmul(o2_ps[:], oT[:, e, :], wo_sb[:, e, :],
                                 start=(e == 0), stop=(e == ET - 1))
            o2 = tl.tile([NQ, E], F32)
            nc.vector.tensor_add(o2[:], o2_ps[:], smalls["bg"][:])

            z = layernorm(nc, tl, o2, smalls["g2r"], smalls["be2r"], eps_sb[:])
            z_bf = tl.tile([NQ, E], BF16)
            nc.vector.tensor_copy(z_bf[:], z[:])
            zT = tl.tile([128, ET, NQ], BF16)
            for e in range(ET):
                tp = tlps.tile([128, NQ], BF16, tag="ps")
                nc.tensor.transpose(tp[:], z_bf[:, e * 128:(e + 1) * 128],
                                    id_bf[:NQ, :NQ])
                nc.scalar.copy(zT[:, e, :], tp[:])
            y1r = tl.tile([128, FT, NQ], BF16)
            for ft in range(FT):
                yp = tlps.tile([128, NQ], F32, tag="ps")
                for e in range(ET):
                    nc.tensor.matmul(
                        yp[:], w1_sb[:, e, ft * 128:(ft + 1) * 128],
                        zT[:, e, :], start=(e == 0), stop=(e == ET - 1))
                nc.scalar.activation(out=y1r[:, ft, :], in_=yp[:], func=AF.Relu,
                                     bias=b1t_sb[:, ft:ft + 1])
            y2_ps = tlps.tile([NQ, E], F32, tag="ps")
            for ft in range(FT):
                nc.tensor.matmul(y2_ps[:], y1r[:, ft, :], w2_sb[:, ft, :],
                                 start=(ft == 0), stop=(ft == FT - 1))
            res = tl.tile([NQ, E], F32)
            nc.vector.tensor_add(res[:], y2_ps[:], o2[:])
            nc.vector.tensor_add(res[:], res[:], smalls["b2r"][:])
            nc.sync.dma_start(out[:, :], res[:])

    _split_multi_waits(nc)
    return nc


def _host_prep(bg, feature_values, feature_batch_idx, feature_level_idx,
               Wq, Wkv, Wo, rope_freqs, ln_attn_g, ln_attn_b,
               ln_ffn_g, ln_ffn_b, W1, b1, W2, b2, max_len):
    """Index/layout-only host prep -> per-core in_maps."""
    bg = np.asarray(bg, np.float32)
    fv = np.ascontiguousarray(np.asarray(feature_values, np.float32))
    lvl = np.asarray(feature_level_idx).astype(np.int64)
    Wq = np.asarray(Wq, np.float32)
    Wkv = np.asarray(Wkv, np.float32)
    Wo = np.asarray(Wo, np.float32)
    freqs = np.asarray(rope_freqs, np.float32)      # [H, D/2]
    W1 = np.asarray(W1, np.float32)
    W2 = np.asarray(W2, np.float32)
    b1 = np.asarray(b1, np.float32)
    b2 = np.asarray(b2, np.float32)

    wk = np.ascontiguousarray(Wkv[:, :E])
    wv = np.ascontiguousarray(Wkv[:, E:])
    wkT = np.ascontiguousarray(wk.T)

    # per-e rope table entries
    e_idx = np.arange(E)
    h_of_e = e_idx // D
    p_of_e = (e_idx % D) // 2
    f_of_e = freqs[h_of_e, p_of_e]                  # [E]
    sign = np.where(e_idx % 2 == 0, -1.0, 1.0).astype(np.float32)

    ind = np.zeros((NLVL, 128), bf16_np)
    for L in range(NLVL):
        ind[L, L * 32:(L + 1) * 32] = 1.0
    selm = np.zeros((128, 32), bf16_np)
    for L in range(NLVL):
        selm[L * 32:(L + 1) * 32, :] = np.eye(32, dtype=bf16_np)

    b1t = np.ascontiguousarray(b1.reshape(FFN // 128, 128).T).astype(np.float32)

    def rep4(v):
        return np.ascontiguousarray(
            np.broadcast_to(np.asarray(v, np.float32)[None, :], (NQ, E)))

    shared = {
        "ind": ind, "sel": selm,
        "wq": Wq.astype(bf16_np), "wkT": wkT.astype(bf16_np),
        "wv": wv.astype(bf16_np), "wo": Wo.astype(bf16_np),
        "w1": W1.astype(bf16_np), "w2": W2.astype(bf16_np), "b1t": b1t,
        "b2r": rep4(b2), "g1r": rep4(ln_attn_g), "be1r": rep4(ln_attn_b),
        "g2r": rep4(ln_ffn_g), "be2r": rep4(ln_ffn_b),
    }

    in_maps = []
    for b in range(BSZ):
        lv = lvl[b * NKEY:(b + 1) * NKEY]
        maskT = np.where(lv[None, :] == np.arange(NLVL)[:, None],
                         np.float32(0.0), np.float32(NEG)).astype(bf16_np)
        qpos = float(b // 2)
        cs = np.empty((NQ, 2 * NLVL, E), np.float32)
        for L in range(NLVL):
            ang = (qpos - L) * f_of_e
            cs[:, L, :] = np.cos(ang)[None, :]
            cs[:, NLVL + L, :] = (sign * np.sin(ang))[None, :]
        cs = cs.reshape(NQ, 2 * NLVL * E)
        m = {
            "fv": fv[b * NKEY:(b + 1) * NKEY],
            "bg": np.ascontiguousarray(bg[b]),
            "maskT": maskT,
            "cs": cs,
        }
        m.update(shared)
        in_maps.append(m)
    return in_maps


def _split_multi_waits(nc):
    """This walrus build caps sync-wait commands at 1 per instruction; Tile
    emits more.  Hoist extra waits onto injected same-engine NOPs placed
    immediately before the waiting instruction (same per-engine order, so
    semantics are identical)."""
    k = 0
    for f in nc.m.functions:
        for bb in f.blocks:
            new_list = []
            for ins in bb.instructions:
                si = ins.sync_info
                if si is not None and si.on_wait and len(si.on_wait) > 1:
                    waits = list(si.on_wait)
                    for w in waits[:-1]:
                        nop = mybir.InstNoOp(
                            name=f"I-sw{k}", ins=[], outs=[], nofuse=True)
                        k += 1
                        nop.engine = ins.engine
                        nop.sync_info = mybir.SyncInfo(
                            on_wait=[w], on_update=[])
                        new_list.append(nop)
                    si.on_wait.clear()
                    si.on_wait.append(waits[-1])
                new_list.append(ins)
            bb.instructions[:] = new_list

_NC_CACHE = None


def _get_nc():
    global _NC_CACHE
    if _NC_CACHE is None:
        _NC_CACHE = _build_bass()
    return _NC_CACHE


def _run(inputs, trace=False):
    nc = _get_nc()
    in_maps = _host_prep(**inputs)
    res = bass_utils.run_bass_kernel_spmd(
        nc, in_maps, core_ids=list(range(BSZ)), trace=trace)
    out = np.stack([res.results[b]["o"] for b in range(BSZ)], axis=0)
    return out.astype(np.float32), res


def kernel(**inputs) -> np.ndarray:
    out, _ = _run(inputs, trace=False)
    return out
